# revision 6
# baseline (speedup 1.0000x reference)
"""Trainium2 Bass kernel for masked causal attention with RoPE (mgdt column masking).

Reference computation (B=4, T=2048, H=512, heads=8, D=64):
  q/k/v = x @ W + b;  RoPE(q, k) over full hidden dim (pairs of adjacent channels);
  scores = q k^T / sqrt(D) with causal tril mask plus fully-masked columns
  at {4, 7, 10, ...} (period 3); softmax; out = (att @ v) @ Wo + bo.

Sharding: 8 cores = data-parallel over batch (4) x tensor-parallel over head
groups (2 x 4 heads). Each core computes a [T, H] partial of its batch's
output projection (Wo row-sharded); host sums the pair of partials + bo.

Device-side layout choices (all matmul operands live in their natural
layouts; no on-device transposes):
  - q/k are computed TRANSPOSED as qT[c, t] = (Wq^T x^T), c on partitions.
  - RoPE needs channel-pair swaps (cross-partition): computed as a second
    projection with column-swapped weights (qsw = x @ Wq_swapped), then
    qrot = q*C + qsw*S elementwise with per-(channel,t) cos/sin tables.
  - Biases enter via an appended ones-row in x^T and a bias row in W.
  - scores are computed transposed (sT[s, t] = K Q^T per head, K=64
    contraction, two heads packed into the 128-row PE array via base
    partitions 0/64) so that softmax's s-reduction and att@V's s-contraction
    both have s on partitions -> no transposes anywhere.
  - p = exp(sT/8) (no max subtraction: |scores/8| < ~2 by construction).
  - V is augmented per head with a ones column -> att@V matmul emits the
    softmax denominator as row 64 of its PSUM output for free. The masked
    columns (ret_cols) are handled by zeroing V rows AND the ones column
    (tensor_scalar by per-partition colmask), so they drop out of both
    numerator and denominator.
  - Normalization: denominators are gathered (DMA), reciprocal'd in one
    [128, 8] DVE op, scattered back to rows, broadcast over partitions with
    a K=2 selector matmul, and applied with one tensor_tensor multiply.
"""

import sys

if "/opt/trn_rl_repo" not in sys.path:
    sys.path.insert(0, "/opt/trn_rl_repo")

import numpy as np
import ml_dtypes

B, T, H, NH, D = 4, 2048, 512, 8, 64
THETA = 10000.0
PERIOD, RET_ORDER = 3, 2
NCORES = 8
CPG = H // 2          # 256 channels per head-group shard
CHUNK = 512           # t-chunk (one PSUM bank of fp32)
NCH = T // CHUNK      # 4 chunks
NST = T // 128        # 16 s-tiles
BF = ml_dtypes.bfloat16

_prog = None


def _build_program():
    global _prog
    if _prog is not None:
        return _prog
    from contextlib import ExitStack
    import concourse.bacc as bacc
    import concourse.tile as tile
    from concourse import mybir

    bf = mybir.dt.bfloat16
    f32 = mybir.dt.float32
    EXP = mybir.ActivationFunctionType.Exp

    nc = bacc.Bacc("TRN2", target_bir_lowering=False, debug=False, num_devices=NCORES)

    def din(name, shape, dt):
        return nc.dram_tensor(name, shape, dt, kind="ExternalInput").ap()

    xt_d = din("xt", [H + 1, T], bf)
    wq_d = din("wq", [H + 1, CPG], bf)
    wqs_d = din("wqs", [H + 1, CPG], bf)
    wk_d = din("wk", [H + 1, CPG], bf)
    wks_d = din("wks", [H + 1, CPG], bf)
    wv_d = din("wv", [H + 1, 260], bf)
    wo_d = din("wo", [CPG, H], bf)
    cos_d = din("cosc", [CPG, T], bf)
    sin_d = din("sins", [CPG, T], bf)
    tril_d = din("tril", [128, 128], bf)
    colm_d = din("colmask", [128, NST], f32)
    sel_d = din("sel", [2, 128], f32)
    bq_d = din("bq", [128, 2], f32)
    bqs_d = din("bqs", [128, 2], f32)
    bk_d = din("bk", [128, 2], f32)
    bks_d = din("bks", [128, 2], f32)
    out_d = nc.dram_tensor("out", [T, H], f32, kind="ExternalOutput").ap()

    with tile.TileContext(nc) as tc:
        with ExitStack() as ctx:
            sg = ctx.enter_context(tc.tile_pool(name="sg", bufs=1))

            # ---- constant / persistent loads ----
            xt_sb = []
            for kt in range(4):
                tl = sg.tile([128, T], bf, tag=f"xt{kt}")
                nc.sync.dma_start(out=tl, in_=xt_d[128 * kt:128 * (kt + 1), :])
                xt_sb.append(tl)
            xt1 = sg.tile([1, T], bf, tag="xt_ones")
            nc.sync.dma_start(out=xt1, in_=xt_d[H:H + 1, :])

            def load_w(dram, cols, tag, bias_row=False):
                tiles = []
                for kt in range(4):
                    tl = sg.tile([128, cols], bf, tag=f"{tag}{kt}")
                    nc.sync.dma_start(out=tl, in_=dram[128 * kt:128 * (kt + 1), :])
                    tiles.append(tl)
                if not bias_row:
                    return tiles, None
                tb = sg.tile([1, cols], bf, tag=f"{tag}b")
                nc.sync.dma_start(out=tb, in_=dram[H:H + 1, :])
                return tiles, tb

            wq_sb, _ = load_w(wq_d, CPG, "wq")
            wqs_sb, _ = load_w(wqs_d, CPG, "wqs")
            wk_sb, _ = load_w(wk_d, CPG, "wk")
            wks_sb, _ = load_w(wks_d, CPG, "wks")
            wv_sb, wv_b = load_w(wv_d, 260, "wv", bias_row=True)
            bias_sb = {}
            for nm, dd in (("bq", bq_d), ("bqs", bqs_d), ("bk", bk_d), ("bks", bks_d)):
                tl = sg.tile([128, 2], f32, tag=nm, name=f"b_{nm}")
                nc.sync.dma_start(out=tl, in_=dd[:, :])
                bias_sb[nm] = tl

            wo_sb = []
            for i in range(2):
                tl = sg.tile([128, H], bf, tag=f"wo{i}")
                nc.sync.dma_start(out=tl, in_=wo_d[128 * i:128 * (i + 1), :])
                wo_sb.append(tl)
            cos_sb, sin_sb = [], []
            for i in range(2):
                tl = sg.tile([128, T], bf, tag=f"cos{i}")
                nc.sync.dma_start(out=tl, in_=cos_d[128 * i:128 * (i + 1), :])
                cos_sb.append(tl)
                tl = sg.tile([128, T], bf, tag=f"sin{i}")
                nc.sync.dma_start(out=tl, in_=sin_d[128 * i:128 * (i + 1), :])
                sin_sb.append(tl)
            tril_sb = sg.tile([128, 128], bf, tag="tril")
            nc.sync.dma_start(out=tril_sb, in_=tril_d[:, :])
            colm_sb = sg.tile([128, NST], f32, tag="colm")
            nc.sync.dma_start(out=colm_sb, in_=colm_d[:, :])
            sel_sb = sg.tile([2, 128], f32, tag="sel")
            nc.sync.dma_start(out=sel_sb, in_=sel_d[:, :])

            # persistent activations
            qrot = {}
            krot = {}
            aot = {}
            for ct in range(2):
                for ch in range(NCH):
                    qrot[ct, ch] = sg.tile([128, CHUNK], bf, tag=f"qr{ct}_{ch}", name=f"qr{ct}_{ch}")
                    krot[ct, ch] = sg.tile([128, CHUNK], bf, tag=f"kr{ct}_{ch}", name=f"kr{ct}_{ch}")
                    aot[ct, ch] = sg.tile([128, CHUNK], bf, tag=f"ao{ct}_{ch}", name=f"ao{ct}_{ch}")
            vaug = []
            for s in range(NST):
                vaug.append(sg.tile([128, 260], bf, tag=f"va{s}", name=f"va{s}"))

            # ---- phase B: projections + rope ----
            with tc.tile_pool(name="ppj", bufs=6, space="PSUM") as ppj, \
                 tc.tile_pool(name="ppv", bufs=2, space="PSUM") as ppv, \
                 tc.tile_pool(name="rtmp", bufs=4) as rtmp:
                ADD = mybir.AluOpType.add
                MULT = mybir.AluOpType.mult
                for ct in range(2):
                    csl = slice(128 * ct, 128 * ct + 128)
                    for (wt, bn, wst, bsn, dst) in (
                        (wq_sb, "bq", wqs_sb, "bqs", qrot),
                        (wk_sb, "bk", wks_sb, "bks", krot),
                    ):
                        for ch in range(NCH):
                            tsl = slice(CHUNK * ch, CHUNK * (ch + 1))
                            pm = ppj.tile([128, CHUNK], f32, tag="pj")
                            pms = ppj.tile([128, CHUNK], f32, tag="pj")
                            for kt in range(4):
                                nc.tensor.matmul(pm, lhsT=wt[kt][:, csl],
                                                 rhs=xt_sb[kt][:, tsl],
                                                 start=(kt == 0), stop=(kt == 3))
                            for kt in range(4):
                                nc.tensor.matmul(pms, lhsT=wst[kt][:, csl],
                                                 rhs=xt_sb[kt][:, tsl],
                                                 start=(kt == 0), stop=(kt == 3))
                            t1 = rtmp.tile([128, CHUNK], f32, tag="t1")
                            t2 = rtmp.tile([128, CHUNK], f32, tag="t2")
                            nc.vector.scalar_tensor_tensor(
                                out=t1, in0=pm, scalar=bias_sb[bn][:, ct:ct + 1],
                                in1=cos_sb[ct][:, tsl], op0=ADD, op1=MULT)
                            nc.vector.scalar_tensor_tensor(
                                out=t2, in0=pms, scalar=bias_sb[bsn][:, ct:ct + 1],
                                in1=sin_sb[ct][:, tsl], op0=ADD, op1=MULT)
                            nc.gpsimd.tensor_add(dst[ct, ch], t1, t2)
                # v projection (+ colmask zeroing, ones column -> denominator)
                for s in range(NST):
                    ssl = slice(128 * s, 128 * (s + 1))
                    pv = ppv.tile([128, 260], f32, tag="pv")
                    for kt in range(4):
                        nc.tensor.matmul(pv, lhsT=xt_sb[kt][:, ssl], rhs=wv_sb[kt],
                                         start=(kt == 0), stop=False)
                    nc.tensor.matmul(pv, lhsT=xt1[0:1, ssl], rhs=wv_b,
                                     start=False, stop=True)
                    nc.vector.tensor_scalar_mul(vaug[s], pv, colm_sb[:, s:s + 1])

            # ---- phase C (attention) + phase D (output projection) ----
            with tc.tile_pool(name="pps", bufs=2, space="PSUM") as pps, \
                 tc.tile_pool(name="ppo", bufs=2, space="PSUM") as ppo, \
                 tc.tile_pool(name="pprd", bufs=1, space="PSUM") as pprd, \
                 tc.tile_pool(name="ppout", bufs=1, space="PSUM") as ppout, \
                 tc.tile_pool(name="pp", bufs=6) as pp, \
                 tc.tile_pool(name="dn", bufs=2) as dn, \
                 tc.tile_pool(name="stg", bufs=2) as stg, \
                 tc.tile_pool(name="ost", bufs=3) as ost:
                for j in range(NCH):
                    for hp in range(2):
                        ct = hp
                        po = [ppo.tile([65, CHUNK], f32, tag="po", name=f"po{j}_{hp}_{i}") for i in range(2)]
                        nst = 4 * j + 4
                        for s in range(nst):
                            r = s - 4 * j
                            col0 = 128 * r if r >= 0 else 0
                            ksl = slice(128 * (s % 4), 128 * (s % 4) + 128)
                            ps = pps.tile([128, 2 * CHUNK], f32, tag="ps")
                            for idx in range(2):
                                pb = 64 * idx
                                nc.tensor.matmul(
                                    ps[:, CHUNK * idx + col0:CHUNK * (idx + 1)],
                                    lhsT=krot[ct, s // 4][pb:pb + 64, ksl],
                                    rhs=qrot[ct, j][pb:pb + 64, col0:],
                                    start=True, stop=True)
                            pt = pp.tile([128, 2 * CHUNK], bf, tag="p")
                            nc.scalar.activation(out=pt[:, col0:], in_=ps[:, col0:],
                                                 func=EXP, scale=0.125)
                            if r >= 0:
                                for idx in range(2):
                                    c0 = CHUNK * idx + col0
                                    nc.gpsimd.tensor_mul(pt[:, c0:c0 + 128],
                                                         pt[:, c0:c0 + 128],
                                                         tril_sb)
                            for idx in range(2):
                                hh = 2 * hp + idx
                                nc.tensor.matmul(
                                    po[idx][:, col0:],
                                    lhsT=vaug[s][:, 65 * hh:65 * hh + 65],
                                    rhs=pt[:, CHUNK * idx + col0:CHUNK * (idx + 1)],
                                    start=(s == 0), stop=(s == nst - 1),
                                    skip_group_check=True)
                        # move PSUM results to SBUF (DMA cannot read PSUM)
                        oA = stg.tile([65, CHUNK], f32, tag="oA")
                        oB = stg.tile([65, CHUNK], f32, tag="oB")
                        nc.vector.tensor_copy(out=oA, in_=po[0])
                        nc.vector.tensor_copy(out=oB, in_=po[1])
                        # head B shifted to partitions 64:128 via SBUF->SBUF DMA
                        shb = stg.tile([128, CHUNK], f32, tag="shb")
                        nc.sync.dma_start(out=shb[64:128, :], in_=oB[0:64, :])
                        # denominators -> reciprocal -> row scatter -> broadcast
                        dsb = dn.tile([128, 8], f32, tag="den")
                        nc.sync.dma_start(out=dsb[:, 0:4], in_=oA[64:65, :])
                        nc.sync.dma_start(out=dsb[:, 4:8], in_=oB[64:65, :])
                        rsb = dn.tile([128, 8], f32, tag="rden")
                        nc.vector.reciprocal(rsb, dsb)
                        rdr = dn.tile([2, CHUNK], f32, tag="rdr")
                        nc.sync.dma_start(out=rdr[0:1, :], in_=rsb[:, 0:4])
                        nc.sync.dma_start(out=rdr[1:2, :], in_=rsb[:, 4:8])
                        prd = pprd.tile([128, CHUNK], f32, tag="prd")
                        nc.tensor.matmul(prd, lhsT=sel_sb, rhs=rdr,
                                         start=True, stop=True)
                        nc.vector.tensor_mul(aot[ct, j][0:64, :], oA[0:64, :],
                                             prd[0:64, :])
                        nc.vector.tensor_mul(aot[ct, j][64:128, :], shb[64:128, :],
                                             prd[64:128, :])
                    # output projection for this chunk's 4 t-tiles
                    for tt in range(4):
                        pout = ppout.tile([128, H], f32, tag="pout")
                        for ct2 in range(2):
                            nc.tensor.matmul(pout,
                                             lhsT=aot[ct2, j][:, 128 * tt:128 * (tt + 1)],
                                             rhs=wo_sb[ct2],
                                             start=(ct2 == 0), stop=(ct2 == 1))
                        osb = ost.tile([128, H], f32, tag="ost")
                        nc.vector.tensor_copy(out=osb, in_=pout)
                        nc.sync.dma_start(
                            out=out_d[CHUNK * j + 128 * tt:CHUNK * j + 128 * (tt + 1), :],
                            in_=osb)

    nc.compile()
    _prog = nc
    return nc


def _host_inputs(x, Wq, bq, Wk, bk, Wv, bv, Wo, bo):
    """Build the 8 per-core input maps (all hardcoded shapes)."""
    x = np.asarray(x, np.float32)
    Wq, bq = np.asarray(Wq, np.float32), np.asarray(bq, np.float32)
    Wk, bk = np.asarray(Wk, np.float32), np.asarray(bk, np.float32)
    Wv, bv = np.asarray(Wv, np.float32), np.asarray(bv, np.float32)
    Wo = np.asarray(Wo, np.float32)

    # x^T with appended ones row, per batch
    xt_all = []
    for b in range(B):
        xt = np.empty((H + 1, T), np.float32)
        xt[:H] = x[b].T
        xt[H] = 1.0
        xt_all.append(xt.astype(BF))

    # rope tables (match reference fp32 math)
    inv = (1.0 / (THETA ** (np.arange(0, H, 2, dtype=np.float32) / H))).astype(np.float32)
    tpos = np.arange(T, dtype=np.float32)
    ang = tpos[:, None] * inv[None, :]          # (T, 256)
    cosf = np.cos(ang).astype(np.float32).T     # (256, T)
    sinf = np.sin(ang).astype(np.float32).T

    swap = np.arange(CPG)
    swap = swap + 1 - 2 * (swap % 2)            # [1,0,3,2,...]

    per_g = []
    for g in range(2):
        cols = slice(CPG * g, CPG * (g + 1))
        wq_a = np.vstack([Wq[:, cols], bq[cols][None]])
        wk_a = np.vstack([Wk[:, cols], bk[cols][None]])
        wv_a = np.zeros((H + 1, 260), np.float32)
        for hh in range(4):
            wv_a[:H, 65 * hh:65 * hh + 64] = Wv[:, CPG * g + 64 * hh:CPG * g + 64 * (hh + 1)]
            wv_a[H, 65 * hh:65 * hh + 64] = bv[CPG * g + 64 * hh:CPG * g + 64 * (hh + 1)]
            wv_a[H, 65 * hh + 64] = 1.0
        pr = slice(128 * g, 128 * (g + 1))
        cos_g = np.repeat(cosf[pr], 2, axis=0)  # (256, T)
        sin_g = np.repeat(sinf[pr], 2, axis=0)
        sin_g = sin_g.copy()
        sin_g[0::2] *= -1.0
        bq_g = bq[cols].reshape(2, 128).T.copy()
        bk_g = bk[cols].reshape(2, 128).T.copy()
        bqs_g = bq[cols][swap].reshape(2, 128).T.copy()
        bks_g = bk[cols][swap].reshape(2, 128).T.copy()
        per_g.append(dict(
            bq=bq_g.astype(np.float32), bqs=bqs_g.astype(np.float32),
            bk=bk_g.astype(np.float32), bks=bks_g.astype(np.float32),
            wq=wq_a.astype(BF), wqs=wq_a[:, swap].astype(BF),
            wk=wk_a.astype(BF), wks=wk_a[:, swap].astype(BF),
            wv=wv_a.astype(BF), wo=Wo[cols, :].astype(BF),
            cosc=np.ascontiguousarray(cos_g).astype(BF),
            sins=np.ascontiguousarray(sin_g).astype(BF),
        ))

    ls, lt = np.meshgrid(np.arange(128), np.arange(128), indexing="ij")
    tril = (ls <= lt).astype(BF)                # allowed where s <= t
    colmask = np.ones(T, np.float32)
    colmask[PERIOD + RET_ORDER - 1::PERIOD] = 0.0
    colm = colmask.reshape(NST, 128).T.copy()   # [128, 16]
    sel = np.zeros((2, 128), np.float32)
    sel[0, :64] = 1.0
    sel[1, 64:] = 1.0

    in_maps = []
    for c in range(NCORES):
        b, g = c // 2, c % 2
        m = dict(xt=xt_all[b], tril=tril, colmask=colm, sel=sel)
        m.update(per_g[g])
        in_maps.append(m)
    return in_maps


def run(inputs, trace=False):
    """Build+run; returns (per-core results list, BassKernelResults)."""
    from concourse.bass_utils import run_bass_kernel_spmd
    nc = _build_program()
    in_maps = _host_inputs(**inputs)
    res = run_bass_kernel_spmd(nc, in_maps, list(range(NCORES)), trace=trace)
    return res


def kernel(x, Wq, bq, Wk, bk, Wv, bv, Wo, bo):
    res = run(dict(x=x, Wq=Wq, bq=bq, Wk=Wk, bk=bk, Wv=Wv, bv=bv, Wo=Wo, bo=bo))
    bo = np.asarray(bo, np.float32)
    out = np.empty((B, T, H), np.float32)
    for b in range(B):
        out[b] = res.results[2 * b]["out"] + res.results[2 * b + 1]["out"] + bo[None, :]
    return out
